# revision 7
# baseline (speedup 1.0000x reference)
"""MLA attention kernel for TRN2, SPMD over 8 NeuronCores (v2).

Sharding: core c = 4*b + g  (b = batch 0..1, g = head-group 0..3, 4 heads each).
Each core computes, for its batch b and head-group g:
    qT = (Wq_g*scale*A)^T x^T + bq_g*scale*A   [256, 2048]   (bf16; A=128/ln2 folded
                                                for fast-exp; undone at exp)
    latT = Wl^T x^T + bl                        [256, 2048]
    kT = Wk_g^T latT                            [256, 2048]   (bk dropped: softmax shift-invariant)
    v  = latT^T Wv_g                            [2048, 256]   (bv folded into host const)
    per head h: sT = kT_h^T qT_h ; pT = exp(sT/A)
                exp split between ScalarE (table exp) and VectorE
                (Schraudolph: bits(bf16) = sT + B, written via int16 cast)
    l = 1^T pT ; oT = v_h^T pT ; aT = oT * approx(1/l)
    partial = A Wo_g                            [2048, 1024]  (bf16 out)
Host sums the 4 partials per batch (f32) and adds (bv @ Wo + bo).
"""
import contextlib
import ctypes
import os
import sys
import types

if "/opt/trn_rl_repo" not in sys.path:
    sys.path.insert(0, "/opt/trn_rl_repo")

import numpy as np
import ml_dtypes

NPBF16 = ml_dtypes.bfloat16
SCALE = 64 ** -0.5
EXP_A = 128.0 / float(np.log(2.0))        # score pre-scale folded into Wq
EXP_INV_A = float(np.log(2.0)) / 128.0    # undo on ScalarE exp
SCHRAUD_B = 127.0 * 128.0 - 0.0579 * 128.0  # mean-centered Schraudolph bias
_STATE = {}

# routing knobs
DVE_EXP_GIS = (1, 2, 4, 6)  # gi groups whose exp runs on VectorE (fast-exp)
DVE_OB_US = (0, 1)          # wo output-chunk u indices copied by VectorE


# ---------------------------------------------------------------- ntff shim
def _install_ntff_shim():
    """Provide antenv.axon_hooks so run_bass_kernel_spmd(trace=True) works."""
    if "antenv.axon_hooks" in sys.modules:
        return
    try:
        import antenv
    except ImportError:
        return

    so_path = "/opt/axon/libaxon_pjrt.so"

    def _hook_factory():
        try:
            lib = ctypes.CDLL(so_path)
        except OSError:
            return None
        if not hasattr(lib, "axon_start_nrt_profile"):
            return None
        lib.axon_start_nrt_profile.argtypes = [ctypes.POINTER(ctypes.c_int64), ctypes.c_size_t]
        lib.axon_start_nrt_profile.restype = ctypes.c_int64
        lib.axon_stop_nrt_profile.argtypes = [ctypes.c_char_p]
        lib.axon_stop_nrt_profile.restype = ctypes.c_int64

        @contextlib.contextmanager
        def _hook(output_dir, device_ids):
            import jax

            jax.devices()
            if device_ids:
                ids = (ctypes.c_int64 * len(device_ids))(*device_ids)
                rc = lib.axon_start_nrt_profile(ids, len(device_ids))
            else:
                rc = lib.axon_start_nrt_profile(None, 0)
            if rc != 0:
                raise RuntimeError(f"axon_start_nrt_profile rc={rc}")
            try:
                yield
            finally:
                n = lib.axon_stop_nrt_profile(str(output_dir).encode())
                print(f"profile: {n} file(s) written to {output_dir}", file=sys.stderr)

        return _hook

    import antenv

    mod = types.ModuleType("antenv.axon_hooks")
    _state = {"hook": _hook_factory()}
    mod.set_axon_ntff_profile_hook = lambda h: _state.__setitem__("hook", h)
    mod.get_axon_ntff_profile_hook = lambda: _state["hook"]
    sys.modules["antenv.axon_hooks"] = mod
    antenv.axon_hooks = mod


# ---------------------------------------------------------------- bass build
def _build_nc(debug_dump=False):
    import concourse.bass as bass  # noqa: F401
    import concourse.tile as tile
    from concourse import bacc, mybir

    BF16 = mybir.dt.bfloat16
    F32 = mybir.dt.float32
    I16 = mybir.dt.int16
    EXP = mybir.ActivationFunctionType.Exp
    IDENT = mybir.ActivationFunctionType.Identity
    ADD = mybir.AluOpType.add

    nc = bacc.Bacc(None, target_bir_lowering=False, debug=False)

    xT = nc.dram_tensor("xT", [128, 4, 8, 512], BF16, kind="ExternalInput")
    wq = nc.dram_tensor("wq", [128, 8, 256], BF16, kind="ExternalInput")
    bq = nc.dram_tensor("bq", [128, 2], F32, kind="ExternalInput")
    wl = nc.dram_tensor("wl", [128, 8, 256], BF16, kind="ExternalInput")
    bl = nc.dram_tensor("bl", [128, 2], F32, kind="ExternalInput")
    wk = nc.dram_tensor("wk", [128, 2, 256], BF16, kind="ExternalInput")
    wv = nc.dram_tensor("wv", [128, 2, 256], BF16, kind="ExternalInput")
    wo = nc.dram_tensor("wo", [128, 2, 1024], BF16, kind="ExternalInput")
    out = nc.dram_tensor("out", [2048, 1024], BF16, kind="ExternalOutput")

    GROUPS = [(2 * i, 2 * i + 2) for i in range(8)]
    GLEN = 2

    with nc.allow_low_precision("bf16 intermediates by design"), tile.TileContext(nc) as tc:
        with (
            tc.tile_pool(name="wpool", bufs=1) as wpool,
            tc.tile_pool(name="xpool", bufs=1) as xpool,
            tc.tile_pool(name="proj", bufs=1) as proj,
            tc.tile_pool(name="ptp", bufs=36) as ptp,
            tc.tile_pool(name="atp", bufs=8) as atp,
            tc.tile_pool(name="obp", bufs=4) as obp,
            tc.tile_pool(name="rpool", bufs=4) as rpool,
            tc.tile_pool(name="ps", bufs=2, space="PSUM") as ps,
        ):
            # ---------------- constants + inputs
            x_n = [xpool.tile([128, 8, 512], BF16, name=f"x_{n}") for n in range(4)]
            wq_sb = wpool.tile([128, 8, 256], BF16)
            wl_sb = wpool.tile([128, 8, 256], BF16)
            wk_sb = wpool.tile([128, 2, 256], BF16)
            wv_sb = wpool.tile([128, 2, 256], BF16)
            wo_sb = wpool.tile([128, 2, 1024], BF16)
            bq_sb = wpool.tile([128, 2], F32)
            bl_sb = wpool.tile([128, 2], F32)
            ones_sb = wpool.tile([128, 1], BF16)
            ones_k1 = wpool.tile([128, 64], F32)
            nc.vector.memset(ones_sb[:], 1.0)
            nc.vector.memset(ones_k1[:], 1.0)

            nc.sync.dma_start(out=wq_sb[:], in_=wq[:])
            nc.sync.dma_start(out=bq_sb[:], in_=bq[:])
            nc.sync.dma_start(out=wl_sb[:], in_=wl[:])
            nc.sync.dma_start(out=bl_sb[:], in_=bl[:])
            nc.sync.dma_start(out=wk_sb[:], in_=wk[:])
            nc.sync.dma_start(out=wv_sb[:], in_=wv[:])
            for n in range(4):
                nc.sync.dma_start(out=x_n[n][:], in_=xT[:, n, :, :])
            nc.sync.dma_start(out=wo_sb[:], in_=wo[:])

            latT_n = [proj.tile([128, 2, 512], BF16, name=f"latT_{i}") for i in range(4)]
            qT_n = [proj.tile([128, 2, 512], BF16, name=f"qT_{i}") for i in range(4)]
            kT_n = [proj.tile([128, 2, 512], BF16, name=f"kT_{i}") for i in range(4)]
            v_sb = proj.tile([128, 16, 256], BF16)

            def ot_ps(name):
                return ps.tile([128, 512], F32, tag="ot", name=name, bufs=2)

            def misc_ps(name):
                return ps.tile([128, 512], F32, tag="s", name=name, bufs=3)

            # HAM warm-up: dummy matmuls while input DMA is in flight
            warm_sb = wpool.tile([128, 512], BF16)
            nc.vector.memset(warm_sb[:], 0.25)
            warm_ps = ps.tile([128, 512], F32, tag="s", name="warm_ps", bufs=3)
            for i in range(24):
                nc.tensor.matmul(
                    warm_ps[:], warm_sb[:, 0:128], warm_sb[:],
                    start=(i == 0), stop=(i == 23),
                )

            # ---------------- projection emitters (interleaved with attention below)
            def emit_proj_n(n):
                for m in range(2):
                    acc = misc_ps(f"lat_ps_{m}_{n}")
                    for k in range(8):
                        nc.tensor.matmul(
                            acc[:],
                            wl_sb[:, k, 128 * m : 128 * m + 128],
                            x_n[n][:, k, :],
                            start=(k == 0),
                            stop=(k == 7),
                        )
                    nc.scalar.activation(
                        latT_n[n][:, m, :], acc[:], IDENT, bias=bl_sb[:, m : m + 1]
                    )
                for m in range(2):
                    acc = misc_ps(f"kt_ps_{m}_{n}")
                    for k in range(2):
                        nc.tensor.matmul(
                            acc[:],
                            wk_sb[:, k, 128 * m : 128 * m + 128],
                            latT_n[n][:, k, :],
                            start=(k == 0),
                            stop=(k == 1),
                        )
                    nc.vector.tensor_copy(out=kT_n[n][:, m, :], in_=acc[:])

            def emit_v(ts):
                for t in ts:
                    acc = misc_ps(f"v_ps_{t}")
                    for k in range(2):
                        nc.tensor.matmul(
                            acc[:, 0:256],
                            latT_n[t // 4][:, k, 128 * (t % 4) : 128 * (t % 4) + 128],
                            wv_sb[:, k, :],
                            start=(k == 0),
                            stop=(k == 1),
                        )
                    nc.scalar.copy(out=v_sb[:, t, :], in_=acc[:, 0:256])

            def emit_qt(n):
                for m in range(2):
                    acc = misc_ps(f"q_ps_{m}_{n}")
                    for k in range(8):
                        nc.tensor.matmul(
                            acc[:],
                            wq_sb[:, k, 128 * m : 128 * m + 128],
                            x_n[n][:, k, :],
                            start=(k == 0),
                            stop=(k == 7),
                        )
                    nc.scalar.activation(
                        qT_n[n][:, m, :], acc[:], IDENT, bias=bq_sb[:, m : m + 1]
                    )

            # ---------------- attention: batched same-mode runs per (ic, pair)
            def emit_pair(ic, p, fillers=None, selfref=None):
                qTc = qT_n[ic]
                h0, h1 = 2 * p, 2 * p + 1
                ot0 = ot_ps(f"ot_{ic}_{p}")
                pts = []
                if selfref is not None:
                    selfref.append((pts, ot0))
                # QK batch: 64x64 quadrant foursomes (4-way concurrent)
                for gi, (t0, t1) in enumerate(GROUPS):
                    s0 = ps.tile([128, GLEN, 512], F32, tag="s", name=f"s0_{ic}_{p}_{gi}", bufs=3)
                    s1 = ps.tile([128, GLEN, 512], F32, tag="s", name=f"s1_{ic}_{p}_{gi}", bufs=3)
                    for t in range(t0, t1):
                        tt = t - t0
                        kTc = kT_n[t // 4]
                        ksl = slice(128 * (t % 4), 128 * (t % 4) + 128)
                        nc.tensor.matmul(
                            s0[:, tt, :], kTc[0:64, p, ksl], qTc[0:64, p, :],
                            start=True, stop=True,
                        )
                        nc.tensor.matmul(
                            s1[:, tt, :], kTc[64:128, p, ksl], qTc[64:128, p, :],
                            start=True, stop=True,
                        )
                    pt0 = ptp.tile([128, GLEN, 512], BF16, tag="pt", name=f"pt0_{ic}_{p}_{gi}")
                    pt1 = ptp.tile([128, GLEN, 512], BF16, tag="pt", name=f"pt1_{ic}_{p}_{gi}")
                    if gi in DVE_EXP_GIS:
                        nc.vector.tensor_scalar(
                            out=pt0[:].bitcast(I16), in0=s0[:],
                            scalar1=SCHRAUD_B, scalar2=None, op0=ADD,
                        )
                        nc.vector.tensor_scalar(
                            out=pt1[:].bitcast(I16), in0=s1[:],
                            scalar1=SCHRAUD_B, scalar2=None, op0=ADD,
                        )
                    else:
                        nc.scalar.activation(pt0[:], s0[:], EXP, scale=EXP_INV_A)
                        nc.scalar.activation(pt1[:], s1[:], EXP, scale=EXP_INV_A)
                    pts.append((pt0, pt1))
                    if fillers and gi in fillers:
                        fillers[gi]()
                # PV batch: col-tile pairs into separate banks
                for gi, (t0, t1) in enumerate(GROUPS):
                    pt0, pt1 = pts[gi]
                    for t in range(t0, t1):
                        tt = t - t0
                        nc.tensor.matmul(
                            ot0[0:64, :], v_sb[:, t, 64 * h0 : 64 * h0 + 64], pt0[:, tt, :],
                            start=(t == 0), stop=(t == 15), skip_group_check=True,
                        )
                        nc.tensor.matmul(
                            ot0[64:128, :], v_sb[:, t, 64 * h1 : 64 * h1 + 64], pt1[:, tt, :],
                            start=(t == 0), stop=(t == 15), skip_group_check=True,
                        )
                return pts, ot0

            Ls = {}

            def emit_sums_chunk(ic, pair_data, gi):
                if gi == 0:
                    L = misc_ps(f"L_{ic}")
                    nc.vector.memset(L[:], 1.0)
                    Ls[ic] = L
                L = Ls[ic]
                t0, t1 = GROUPS[gi]
                for t in range(t0, t1):
                    tt = t - t0
                    for p in range(2):
                        pt0, pt1 = pair_data[p][0][gi]
                        for hh, pt in ((2 * p, pt0), (2 * p + 1, pt1)):
                            nc.tensor.matmul(
                                L[32 * hh : 32 * hh + 1, :],
                                ones_sb[:],
                                pt[:, tt, :],
                                start=(t == 0),
                                stop=(t == 15),
                                tile_position=(0, 32 * hh),
                                skip_group_check=True,
                            )

            def emit_norm(ic, pair_data):
                L = Ls.pop(ic)
                recip = rpool.tile([128, 512], F32, tag="recip", name=f"recip_{ic}")
                nc.vector.reciprocal_approx_fast(out=recip[:], in_=L[:])
                ats = []
                for p in range(2):
                    bc_ps = misc_ps(f"bcp_{ic}_{p}")
                    for j, hh in enumerate((2 * p, 2 * p + 1)):
                        rb = 32 * hh
                        nc.tensor.matmul(
                            bc_ps[64 * j : 64 * j + 64, :],
                            ones_k1[rb : rb + 1, 0:64],
                            recip[rb : rb + 1, :],
                            start=True,
                            stop=True,
                            tile_position=(rb, 64 * j),
                            skip_group_check=True,
                        )
                    bc = rpool.tile([128, 512], BF16, tag="bc", name=f"bcs_{ic}_{p}")
                    nc.scalar.copy(out=bc[:], in_=bc_ps[:])
                    at = atp.tile([128, 512], BF16, tag="at", name=f"at_{ic}_{p}")
                    ot = pair_data[p][1]
                    nc.vector.tensor_mul(out=at[:], in0=ot[:], in1=bc[:])
                    ats.append(at)
                return ats

            def emit_tail(ic, pair_data):
                for gi in range(8):
                    emit_sums_chunk(ic, pair_data, gi)
                return emit_norm(ic, pair_data)

            def emit_wo_chunk(ic, ats, u):
                ob = obp.tile([128, 2, 512], BF16, tag="ob", name=f"ob_{ic}_{u}")
                for n2 in range(2):
                    wo_ps = ot_ps(f"wo_{ic}_{u}_{n2}")
                    for p in range(2):
                        nc.tensor.matmul(
                            wo_ps[:],
                            ats[p][:, 128 * u : 128 * u + 128],
                            wo_sb[:, p, 512 * n2 : 512 * n2 + 512],
                            start=(p == 0),
                            stop=(p == 1),
                        )
                    if u in DVE_OB_US:
                        nc.vector.tensor_copy(out=ob[:, n2, :], in_=wo_ps[:])
                    else:
                        nc.scalar.copy(out=ob[:, n2, :], in_=wo_ps[:])
                r0 = 512 * ic + 128 * u
                nc.sync.dma_start(out=out[r0 : r0 + 128, :], in_=ob[:])

            def emit_wo(ic, ats):
                for u in range(4):
                    emit_wo_chunk(ic, ats, u)

            emit_qt(0)
            emit_proj_n(0)
            pd00 = emit_pair(0, 0, {
                0: lambda: emit_v(range(0, 4)),
                1: lambda: emit_proj_n(1),
                2: lambda: emit_v(range(4, 8)),
                3: lambda: emit_proj_n(2),
                4: lambda: emit_v(range(8, 12)),
                5: lambda: (emit_proj_n(3), emit_v(range(12, 16))),
            })
            pd01 = emit_pair(0, 1, {1: lambda: emit_qt(1)})
            state = {}
            pd10 = emit_pair(1, 0, {g: (lambda g=g: emit_sums_chunk(0, [pd00, pd01], g)) for g in range(8)})
            pd11 = emit_pair(1, 1, {
                0: lambda: state.__setitem__("a0", emit_norm(0, [pd00, pd01])),
                1: lambda: emit_qt(2),
                2: lambda: emit_wo_chunk(0, state["a0"], 0),
                3: lambda: emit_wo_chunk(0, state["a0"], 1),
                4: lambda: emit_wo_chunk(0, state["a0"], 2),
                5: lambda: emit_wo_chunk(0, state["a0"], 3),
            })
            pd20 = emit_pair(2, 0, {g: (lambda g=g: emit_sums_chunk(1, [pd10, pd11], g)) for g in range(8)})
            pd21 = emit_pair(2, 1, {
                0: lambda: state.__setitem__("a1", emit_norm(1, [pd10, pd11])),
                1: lambda: emit_qt(3),
                2: lambda: emit_wo_chunk(1, state["a1"], 0),
                3: lambda: emit_wo_chunk(1, state["a1"], 1),
                4: lambda: emit_wo_chunk(1, state["a1"], 2),
                5: lambda: emit_wo_chunk(1, state["a1"], 3),
            })
            pd30 = emit_pair(3, 0, {g: (lambda g=g: emit_sums_chunk(2, [pd20, pd21], g)) for g in range(8)})
            pd31_box = []

            def sums3(g):
                emit_sums_chunk(3, [pd30, pd31_box[0]], g)

            pd31 = emit_pair(3, 1, {
                0: lambda: state.__setitem__("a2", emit_norm(2, [pd20, pd21])),
                2: lambda: (emit_wo_chunk(2, state["a2"], 0), sums3(0), sums3(1)),
                3: lambda: (emit_wo_chunk(2, state["a2"], 1), sums3(2)),
                4: lambda: (emit_wo_chunk(2, state["a2"], 2), sums3(3)),
                5: lambda: (emit_wo_chunk(2, state["a2"], 3), sums3(4)),
                6: lambda: sums3(5),
                7: lambda: (sums3(6), sums3(7)),
            }, selfref=pd31_box)
            ats3 = emit_norm(3, [pd30, pd31])
            emit_wo(3, ats3)

    nc.compile()
    return nc


def _get_nc():
    if "nc" not in _STATE:
        _STATE["nc"] = _build_nc()
    return _STATE["nc"]


# ---------------------------------------------------------------- host side
def _pack_k(a, kchunks):
    """[K, N] f32/bf16 -> [128, kchunks, N] bf16 (K = 128*kchunks)."""
    K, N = a.shape
    return np.ascontiguousarray(
        np.asarray(a, np.float32).reshape(kchunks, 128, N).transpose(1, 0, 2)
    ).astype(NPBF16)


def _pack_x(xb):
    """x[b] [2048, 1024] -> xT packed [128, 4, 8, 512] (n-major, 8KB lines)."""
    xT = np.asarray(xb, np.float32).T  # [1024, 2048]
    return np.ascontiguousarray(
        xT.reshape(8, 128, 4, 512).transpose(1, 2, 0, 3)
    ).astype(NPBF16)


def kernel(x, Wq, bq, Wl, bl, Wk, bk, Wv, bv, Wo, bo):
    x = np.asarray(x, np.float32)
    Wq = np.asarray(Wq, np.float32)
    bq = np.asarray(bq, np.float32)
    Wl = np.asarray(Wl, np.float32)
    bl = np.asarray(bl, np.float32)
    Wk = np.asarray(Wk, np.float32)
    Wv = np.asarray(Wv, np.float32)
    bv = np.asarray(bv, np.float32)
    Wo = np.asarray(Wo, np.float32)
    bo = np.asarray(bo, np.float32)

    from concourse.bass_utils import run_bass_kernel_spmd

    trace = os.environ.get("KERNEL_TRACE", "0") == "1"
    if trace:
        _install_ntff_shim()

    qscale = SCALE * EXP_A
    wl_p = _pack_k(Wl, 8)
    bl_p = np.ascontiguousarray(bl.reshape(2, 128).T).astype(np.float32)
    x_p = [_pack_x(x[b]) for b in range(2)]
    in_maps = []
    for c in range(8):
        b, g = divmod(c, 4)
        sl = slice(256 * g, 256 * g + 256)
        in_maps.append(
            {
                "xT": x_p[b],
                "wq": _pack_k(Wq[:, sl] * qscale, 8),
                "bq": np.ascontiguousarray((bq[sl] * qscale).reshape(2, 128).T).astype(np.float32),
                "wl": wl_p,
                "bl": bl_p,
                "wk": _pack_k(Wk[:, sl], 2),
                "wv": _pack_k(Wv[:, sl], 2),
                "wo": _pack_k(Wo[sl, :], 2),
            }
        )

    nc = _get_nc()
    res = run_bass_kernel_spmd(nc, in_maps, core_ids=list(range(8)), trace=trace)
    if trace and res.exec_time_ns is not None:
        print(f"HW exec time: {res.exec_time_ns} ns")
        _STATE["exec_time_ns"] = res.exec_time_ns

    parts = [np.asarray(res.results[c]["out"], np.float32) for c in range(8)]
    const = (bv @ Wo + bo).astype(np.float32)
    out = np.empty((2, 2048, 1024), np.float32)
    for b in range(2):
        out[b] = parts[4 * b] + parts[4 * b + 1] + parts[4 * b + 2] + parts[4 * b + 3] + const
    return out
